# revision 13
# baseline (speedup 1.0000x reference)
"""Trainium2 Bass kernel for nn_Compressor (sparse_attention, hierarchical window MLP).

Reference computation (per batch b, head h):
  windows w=0..510 over k[b,h] (S=8192, D=128), window length 32, stride 16
  x[w, l, :] = k[16w+l, :] + pe[l, :]
  5 stages of pairwise-merge MLP: x <- silu(x.reshape(-1, 256) @ w_down[i].T)
  out[w+1] = x @ w_stop.T   ; out[0] = 0 (prepended zero window)

Sharding: head-parallel across 8 cores (B*H = 32 -> 4 heads/core), weights
replicated, no cross-device comms.

Algebraic optimization: stage-0 operates on adjacent row pairs (s=2t, 2t+1)
and every pair is shared by exactly two windows (stride 16, pair width 2),
always in the same even/odd role.  So
  Z[:, t] = W0_even @ kT[:, 2t] + W0_odd @ kT[:, 2t+1]
is computed once per pair (half the naive stage-0 flops).  The host pre-adds
pe[l] (the use-A positional encoding, position l = s mod 16) into k itself,
so the use-A silu needs no bias; the second use of each pair (use B,
position l+16 in the previous window) differs only by the linear image of
the pe difference,
  dpe0 = W0 @ (pe_B - pe_A)  [128, 8],
folded into the ScalarE activation bias of the second silu.

The kernel is ScalarE(ACT)-bound: every silu element costs one lane-cycle
at 1.2 GHz (~53us/core of pure stream time) plus ~145ns of PSUM-access init
per ACTIVATE instruction, so the design keeps the ACT stream gapless and the
instruction count low:
  - Z planes live in [128, 2, 512] two-bank PSUM tiles so the bias-free
    use-A silu covers two planes per instruction (1022 elems).
  - startup DMAs issue from BOTH HWDGE rings (SP and ACT) in parallel, with
    the critical w0+dpe constants packed into one byte-blob DMA (one
    completion semaphore) and per-plane leading k chunks for head 0.
  - absorber matmuls (2-col, into a region the next start=True matmul
    overwrites) make PE observe each DMA semaphore cheaply; no dummy bank.
  - the stage/stop units of head h interleave with the Z units of head h+1;
    ktf DMAs are hoisted a full phase early so Z never stalls on transfer.
  - the last head has no Z partner, so the use-B silus of its last two Z
    tiles are deferred into its stage-1 PE-fill bubbles, and its stage-3/4
    silus and w_stop chain are split in half so the endgame ping-pongs.

Layout: everything is kept "plane-major" so every matmul moving operand,
every activation input/output, and every copy is contiguous:
  ktp[d, l, w]   = bf16 (k[16w + l, d] + pe[l, d])  -- host provides this
                   fully transposed, so the HBM->SBUF DMA is a straight
                   contiguous copy (no xbar transpose on the device)
  Z tile t       = one [128, 2, 512] psum pair (planes 2t, 2t+1)
  s{i}[d, p, w]  = silu-merged planes, stage i
The final w_stop matmul uses the data as the stationary operand, producing
output already row-major for a clean DMA out per head; its PSUM comes from
the stage pool (the slot stage-3 just freed), so Z + stages use all 8 banks.
"""

import numpy as np

B, H, S, D = 2, 16, 8192, 128
BH = B * H
NCORES = 8
HPC = BH // NCORES  # heads per core = 4
NB = (S - 32) // 16 + 1  # 511 sliding windows
NW = NB + 1  # 512 output rows per head (incl. zero window)

# w_stop output chunking: window ranges per PE (stationary) chunk
QRANGES = [(0, 128), (128, 128), (256, 128), (384, 127)]

# junk matmuls keeping PE at full DVFS pstate through the DMA ramp
WARMUP_MMS = 40

CRIT_BYTES = 2 * 128 * 2 + 8 * 4  # w0 (bf16 [2,128]) + dpe (f32 [8]) = 544
REST_BYTES = 4 * 2 * 128 * 2 + 128 * 2  # w1..4 + wst = 2304

_BASS_CACHE = {}


def _build_bass():
    import concourse.bacc as bacc
    import concourse.mybir as mybir
    import concourse.tile as tile

    f32 = mybir.dt.float32
    bf16 = mybir.dt.bfloat16
    u8 = mybir.dt.uint8
    SILU = mybir.ActivationFunctionType.Silu

    nc = bacc.Bacc()
    # k4p[hh, d, 512*l + w] = bf16(k[16w + l, d] + pe[l, d]): the l-planar
    # transposed layout, prepared on the host so the device DMA is contiguous.
    k4p = nc.dram_tensor("k4p", [HPC, D, S], bf16, kind="ExternalInput")
    # packed constant byte blobs (single DMA + single completion sem each):
    # crit = w0[k, h, o] bf16 | dpe[k, e] f32 ; rest = w1..4[k, i, h, o] | wst
    cstc = nc.dram_tensor("cstc", [128, CRIT_BYTES], u8, kind="ExternalInput")
    cstr = nc.dram_tensor("cstr", [128, REST_BYTES], u8, kind="ExternalInput")
    oqs = [
        nc.dram_tensor(f"o{hh}", [513, 128], f32, kind="ExternalOutput")
        for hh in range(HPC)
    ]

    with tile.TileContext(nc) as tc:
        with (
            tc.tile_pool(name="consts", bufs=1) as consts,
            tc.tile_pool(name="ktp", bufs=2) as ktp,
            tc.tile_pool(name="s0p", bufs=2) as s0p,
            tc.tile_pool(name="stp", bufs=2) as stp,
            tc.tile_pool(name="outp", bufs=2) as outp,
            tc.tile_pool(name="zps", bufs=2, space="PSUM") as zps,
            tc.tile_pool(name="sps", bufs=2, space="PSUM") as sps,
        ):
            csb = consts.tile([128, CRIT_BYTES], u8, name="csb")
            rsb = consts.tile([128, REST_BYTES], u8, name="rsb")
            w0v = csb[:, 0:512].bitcast(bf16).rearrange("p (h o) -> p h o", o=128)
            dpev = csb[:, 512:544].bitcast(f32)  # [128, 8]
            w14 = (
                rsb[:, 0:2048]
                .bitcast(bf16)
                .rearrange("p (i h o) -> p i h o", h=2, o=128)
            )
            wstv = rsb[:, 2048:2304].bitcast(bf16)  # [128, 128]

            def wd(st, h):
                # stage-st (0..4) even/odd half weight, as matmul lhsT
                return w0v[:, h, :] if st == 0 else w14[:, st - 1, h, :]

            # Absorber matmuls: make PE observe a DMA completion semaphore
            # cheaply (the walrus pipeline fuses one wait per LDWEIGHTS slot).
            # Each writes a 2-col sliver of a PSUM region that the next
            # start=True matmul overwrites, so no dedicated bank is needed.
            def absorb(dst2, src):
                return nc.tensor.matmul(
                    dst2, lhsT=src[:, 0:2], rhs=src[:, 0:2], start=True, stop=True
                )

            # ---- startup DMAs, dual-ring ------------------------------------
            # critical consts (w0+dpe) first on the SP ring; head-0 k chunks
            # for planes 0 and 1 on the ACT ring in parallel (ACT is idle
            # until the first silu, and both are HWDGE rings).
            nc.sync.dma_start(out=csb, in_=cstc[:])
            ktf0 = ktp.tile([128, S], bf16, name="ktf")
            nc.scalar.dma_start(out=ktf0[:, 0:1024], in_=k4p[0, :, 0:1024])
            nc.sync.dma_start(out=ktf0[:, 1024:2048], in_=k4p[0, :, 1024:2048])
            nc.sync.dma_start(out=ktf0[:, 2048:4096], in_=k4p[0, :, 2048:4096])
            nc.sync.dma_start(out=rsb, in_=cstr[:])
            nc.sync.dma_start(out=ktf0[:, 4096:6144], in_=k4p[0, :, 4096:6144])
            nc.sync.dma_start(out=ktf0[:, 6144:8192], in_=k4p[0, :, 6144:8192])

            def issue_ktf(hh, split=False):
                ktf = ktp.tile([128, S], bf16, name="ktf")
                if split:
                    # two halves so the first Z tiles don't wait on the
                    # whole 2MB transfer's completion semaphore
                    nc.sync.dma_start(out=ktf[:, 0:4096], in_=k4p[hh, :, 0:4096])
                    nc.sync.dma_start(out=ktf[:, 4096:8192], in_=k4p[hh, :, 4096:8192])
                else:
                    nc.sync.dma_start(out=ktf, in_=k4p[hh])
                return ktf

            s0s = {}

            def z_plane_mms(zz, i, kt3, l0):
                # Z plane into bank i of tile zz from k l-planes l0, l0+1
                nc.tensor.matmul(
                    zz[:, i, :], lhsT=wd(0, 0), rhs=kt3[:, l0, :],
                    start=True, stop=False,
                )
                nc.tensor.matmul(
                    zz[:, i, :], lhsT=wd(0, 1), rhs=kt3[:, l0 + 1, :],
                    start=False, stop=True,
                )

            def silu_a2(s0, t, zz):
                # bias-free use-A silu over both planes of tile t
                nc.scalar.activation(
                    out=s0[:, 2 * t : 2 * t + 2, :], in_=zz[:, :, 0:NB], func=SILU
                )

            def silu_b(s0, e, zz, i):
                # use-B silu of Z plane e (bank i of its tile), dpe bias
                nc.scalar.activation(
                    out=s0[:, 8 + e, :], in_=zz[:, i, 1 : NB + 1], func=SILU,
                    bias=dpev[:, e : e + 1], scale=1.0,
                )

            def z0_units():
                """Head-0 Z phase: latency-critical ramp.

                Planes 0 and 1 run in separate (zps/sps) tiles so the first
                silu needs only the first 1024-col chunk; later planes use
                2-plane tiles with batched use-A silus.  The stage pool is
                idle during this phase, so its banks serve as extra ring
                depth to absorb DMA-completion jitter.
                """
                kt3 = ktf0.rearrange("p (l w) -> p l w", w=512)
                s0 = s0p.tile([128, 16, NB], bf16, name="s0")
                s0s[0] = s0
                # plane 0 (zps tile, bank 0): absorb crit + chunk0 first
                za = zps.tile([128, 2, 512], f32, name="zp", tag="zp")
                # PE warmup: the tensor engine starts at the low DVFS pstate
                # and needs ~3us of continuous execution to reach 2.4GHz.
                # Junk 1-col matmuls on an always-initialized const AP keep it
                # busy from preamble-exit until the first k chunk lands, so
                # the ramp-critical Z matmuls run at full speed.
                warm = nc.const_aps.tensor(1.0, (128, 1), bf16)
                for _ in range(WARMUP_MMS):
                    nc.tensor.matmul(
                        za[0:1, 0, 8:9], lhsT=warm, rhs=warm, start=True, stop=True
                    )
                absorb(za[0:2, 0, 0:2], w0v[:, 0, :])
                absorb(za[0:2, 0, 2:4], kt3[:, 0, :])
                z_plane_mms(za, 0, kt3, 0)
                nc.scalar.activation(out=s0[:, 0:1, :], in_=za[:, 0:1, 0:NB], func=SILU)
                silu_b(s0, 0, za, 0)
                yield
                # plane 1 (sps tile, bank 0)
                zb = sps.tile([128, 2, 512], f32, name="ps", tag="sp")
                absorb(zb[0:2, 0, 0:2], kt3[:, 2, :])
                z_plane_mms(zb, 0, kt3, 2)
                nc.scalar.activation(out=s0[:, 1:2, :], in_=zb[:, 0:1, 0:NB], func=SILU)
                silu_b(s0, 1, zb, 0)
                yield
                # pairs (2,3), (4,5), (6,7) in alternating zps/sps tiles;
                # absorb each 2048-col chunk's sem at the tile that uses it
                chunk_absorb = {1: 4, 2: 8, 3: 12}  # tile t -> first k l-plane
                pools = {1: zps, 2: sps, 3: zps}
                tags = {1: ("zp", "zp"), 2: ("sp", "ps"), 3: ("zp", "zp")}
                for t in range(1, 4):
                    tag, name = tags[t]
                    zz = pools[t].tile([128, 2, 512], f32, name=name, tag=tag)
                    absorb(zz[0:2, 0, 0:2], kt3[:, chunk_absorb[t], :])
                    if t == 2:
                        # rest consts land well before stage-1 needs them
                        absorb(zz[0:2, 0, 2:4], w14[:, 0, 0, :])
                    z_plane_mms(zz, 0, kt3, 4 * t)
                    z_plane_mms(zz, 1, kt3, 4 * t + 2)
                    silu_a2(s0, t, zz)
                    yield
                    silu_b(s0, 2 * t, zz, 0)
                    silu_b(s0, 2 * t + 1, zz, 1)
                    yield

            def z_units(hh, ktf, deferred):
                """Z phase of head hh>=1: four 2-plane tiles.

                When `deferred` is not None (last head), the use-B silus of
                tiles 2 and 3 are pushed into it; stage_units replays them
                in its stage-1 PE-fill bubbles (this head has no Z partner).
                """
                kt3 = ktf.rearrange("p (l w) -> p l w", w=512)
                s0 = s0p.tile([128, 16, NB], bf16, name="s0")
                s0s[hh] = s0
                for t in range(4):
                    zz = zps.tile([128, 2, 512], f32, name="zp", tag="zp")
                    if t == 0 or (hh == 1 and t == 2):
                        absorb(zz[0:2, 0, 0:2], kt3[:, 4 * t, :])
                    z_plane_mms(zz, 0, kt3, 4 * t)
                    z_plane_mms(zz, 1, kt3, 4 * t + 2)
                    silu_a2(s0, t, zz)
                    yield
                    if deferred is not None and t >= 2:
                        deferred.append((s0, 2 * t, zz, 0))
                        deferred.append((s0, 2 * t + 1, zz, 1))
                    else:
                        silu_b(s0, 2 * t, zz, 0)
                        silu_b(s0, 2 * t + 1, zz, 1)
                    yield

            def stage_units(hh, s0, deferred):
                """Stages 1..4 + w_stop of head hh.

                For the last head the deferred use-B silus fill the stage-1
                PE-fill bubbles, stage-3/4 silus are split in half so the
                PE/ACT ping-pong has no full-group bubbles, and the w_stop
                chain is split so the first half's output DMA overlaps the
                second half's compute.
                """
                split_tail = hh == HPC - 1
                prev = s0
                for st in range(1, 5):
                    nj = 16 >> st
                    cur = stp.tile([128, nj, NB], bf16, name=f"s{st}", tag=f"s{st}")
                    for p in range((nj + 1) // 2):
                        npl = min(2, nj - 2 * p)
                        ps = sps.tile([128, 2, 512], f32, name="ps", tag="sp")
                        for ii in range(npl):
                            i = 2 * p + ii
                            nc.tensor.matmul(
                                ps[:, ii, :NB], lhsT=wd(st, 0),
                                rhs=prev[:, 2 * i, :],
                                start=True, stop=False,
                            )
                            nc.tensor.matmul(
                                ps[:, ii, :NB], lhsT=wd(st, 1),
                                rhs=prev[:, 2 * i + 1, :],
                                start=False, stop=True,
                            )
                        if st == 1 and split_tail and deferred:
                            # replay deferred use-B silus while PE fills this
                            # group (2 at g0, 1 at g1, 1 at g2; all must land
                            # before g3's matmuls, which read s0 planes 12-15)
                            for _ in range(2 if p == 0 else 1):
                                if deferred:
                                    silu_b(*deferred.pop(0))
                        if split_tail and st == 4:
                            for j in range(npl):
                                nc.scalar.activation(
                                    out=cur[:, 2 * p + j, 0:256],
                                    in_=ps[:, j, 0:256], func=SILU,
                                )
                                nc.scalar.activation(
                                    out=cur[:, 2 * p + j, 256:NB],
                                    in_=ps[:, j, 256:NB], func=SILU,
                                )
                        else:
                            nc.scalar.activation(
                                out=cur[:, 2 * p : 2 * p + npl, :],
                                in_=ps[:, :npl, :NB], func=SILU,
                            )
                        yield
                    prev = cur

                # w_stop with data-stationary -> row-major [w, o] output; all
                # 4 chunks packed into the first bank of a Z-pool slot: the Z
                # phase of head hh+1 finished earlier in this phase, and its
                # successor's tiles only reach this slot again mid-next-phase,
                # well after the DVE copy drains it (keeping the copy off the
                # stage pool's critical WAR path).
                s4f = prev[:, 0, :]  # [128, 511]
                outsb = outp.tile([128, 4, 128], f32, name="outsb")
                ps2 = zps.tile([128, 2, 512], f32, name="ps2", tag="zp")
                ps2v = ps2.rearrange("p a (q o) -> p a q o", o=128)
                o_view = oqs[hh][1:513].rearrange("(q p) o -> p q o", q=4)
                for q, (w0c, wq) in enumerate(QRANGES):
                    nc.tensor.matmul(
                        ps2v[:wq, 0, q, :],
                        lhsT=s4f[:, w0c : w0c + wq],
                        rhs=wstv,
                        start=True, stop=True,
                    )
                    if split_tail and q == 1:
                        # first half out while the second half computes
                        nc.vector.tensor_copy(
                            out=outsb[:, 0:2, :], in_=ps2v[:, 0, 0:2, :]
                        )
                        nc.sync.dma_start(
                            out=o_view[:, 0:2, :], in_=outsb[:, 0:2, :]
                        )
                if split_tail:
                    nc.vector.tensor_copy(out=outsb[:, 2:4, :], in_=ps2v[:, 0, 2:4, :])
                    nc.sync.dma_start(out=o_view[:, 2:4, :], in_=outsb[:, 2:4, :])
                else:
                    nc.vector.tensor_copy(out=outsb, in_=ps2v[:, 0, :, :])
                    nc.gpsimd.dma_start(out=o_view, in_=outsb)
                yield

            # ---- driver: software pipeline ----------------------------------
            # head-0 Z runs alone (ramp); then the stage/stop units of head h
            # interleave with the Z units of head h+1 so the in-order ACT/PE
            # streams always have independent work to backfill gaps.  ktf
            # DMAs are hoisted one full phase ahead of their Z phase.
            kts = {1: issue_ktf(1, split=True)}
            for _ in z0_units():
                pass
            deferred = []
            for hh in range(HPC):
                gens = [stage_units(hh, s0s.pop(hh), deferred)]
                if hh + 1 < HPC:
                    if hh + 2 < HPC:
                        kts[hh + 2] = issue_ktf(hh + 2)
                    gens.append(
                        z_units(
                            hh + 1,
                            kts.pop(hh + 1),
                            deferred if hh + 1 == HPC - 1 else None,
                        )
                    )
                while gens:
                    for g in list(gens):
                        try:
                            next(g)
                        except StopIteration:
                            gens.remove(g)

    if not nc.is_finalized():
        nc.finalize()
    return nc


def _prep_host_inputs(k, pe, w_down, w_stop):
    import ml_dtypes

    bf16 = ml_dtypes.bfloat16
    k = np.asarray(k, dtype=np.float32)
    pe = np.asarray(pe, dtype=np.float32)
    w_down = np.asarray(w_down, dtype=np.float32)
    w_stop = np.asarray(w_stop, dtype=np.float32)

    # k4p[bh, d, 512*l + w] = k[bh, 16w+l, d] + pe[l, d], cast to bf16 (RNE):
    # the fully transposed l-planar layout with the use-A pe pre-added, so the
    # device-side DMA is a straight contiguous copy
    kr = k.reshape(BH, 512, 16, D) + pe[:16][None, None, :, :]
    k4p = np.ascontiguousarray(kr.transpose(0, 3, 2, 1)).astype(bf16)
    # wdt[k, i, h, o] = w_down[i][o, 128h + k]: matmul lhsT layout
    wdt = np.ascontiguousarray(
        w_down.reshape(5, 128, 2, 128).transpose(3, 0, 2, 1)
    ).astype(bf16)
    # dpe[o, e] = (W0 @ (pe_pair(e+8) - pe_pair(e)))[o]: use-B bias correction
    pe_pairs = pe.reshape(16, 256).astype(np.float64)
    dpe = (
        w_down[0].astype(np.float64) @ (pe_pairs[8:] - pe_pairs[:8]).T
    ).astype(np.float32)
    wst = np.ascontiguousarray(w_stop.T).astype(bf16)

    # pack constants into byte blobs: one DMA + one completion sem each
    cstc = np.empty((128, CRIT_BYTES), dtype=np.uint8)
    cstc[:, 0:512] = wdt[:, 0].reshape(128, 256).view(np.uint8)
    cstc[:, 512:544] = dpe.view(np.uint8)
    cstr = np.empty((128, REST_BYTES), dtype=np.uint8)
    cstr[:, 0:2048] = np.ascontiguousarray(wdt[:, 1:5]).reshape(128, 1024).view(np.uint8)
    cstr[:, 2048:2304] = wst.view(np.uint8)
    return k4p, cstc, cstr


def run(k, pe, w_down, w_stop, trace=False, trace_kwargs=None):
    from concourse.bass_utils import run_bass_kernel_spmd

    k4p, cstc, cstr = _prep_host_inputs(k, pe, w_down, w_stop)

    if "nc" not in _BASS_CACHE:
        _BASS_CACHE["nc"] = _build_bass()
    nc = _BASS_CACHE["nc"]

    in_maps = [
        {
            "k4p": np.ascontiguousarray(k4p[HPC * c : HPC * (c + 1)]),
            "cstc": cstc,
            "cstr": cstr,
        }
        for c in range(NCORES)
    ]
    res = run_bass_kernel_spmd(
        nc, in_maps, core_ids=list(range(NCORES)), trace=trace,
        **(trace_kwargs or {}),
    )
    out = np.empty((BH, NW, D), dtype=np.float32)
    for c in range(NCORES):
        r = res.results[c]
        for hh in range(HPC):
            row = HPC * c + hh
            out[row, 0, :] = 0.0
            out[row, 1:NW, :] = r[f"o{hh}"][1:NW]
    out = out.reshape(B, H, NW, D)
    return out, res


def kernel(k, pe, w_down, w_stop):
    out, _ = run(k, pe, w_down, w_stop, trace=False)
    return out
